# revision 16
# baseline (speedup 1.0000x reference)
"""MoE (DbrxExperts) expert-parallel Trainium2 kernel, v3 (bf16, ft-outer).

Strategy:
  - Host: route tokens to experts, sort experts by count desc, rank-pair
    onto cores (core m: slot0 = rank[m], slot1 = rank[15-m]). Slot
    capacities C0/C1 are exact maxima, baked into the program.
  - All matmul operands bf16 (PE rate identical to fp32r; DMA halved;
    whole slot's xt + hact fit in SBUF). fp32 accumulation in PSUM.
  - Device, per slot (ft-outer so w1/v1 stream exactly once):
      xt (8 tiles [128, C]) DMA'd up front, consumed ht-progressively.
      for ft: load w1s/v1s [128, HT*128]; for each column-pair group:
        gate/up accumulate over ht in PSUM; silu (ACT) * up (DVE) ->
        hact[ft] bf16 in SBUF.
      for ht2 (8 h-tiles): for token-part (<=512): down_T[h, c]
        accumulated over 16 ft in PSUM -> copy -> DMA y [H, C] fp32
        (tokens moving => no 128-padding of the token dim).
  - Host: out[tokens_e] += down_e.T * cw_e.
"""

import numpy as np
from contextlib import ExitStack

N_CORES = 8
B, S, H = 4, 2048, 1024
F, E = 2048, 16
T = B * S
N_SLOTS = 2  # experts per core

P = 128
HT = H // P   # 8  h-tiles
FT = F // P   # 16 f-tiles

TRACE = False          # test.py sets this for profiled runs
TRACE_CORES = [7]      # core-0 NTFF capture crashes fast kernels here
LAST_RESULT = None     # BassKernelResults of last run (for test.py)

_nc_cache = {}


def _parts(s):
    """Split s columns into matmul parts, each <=512 and >=256 where
    possible (moving dim >=256 keeps full PE rate)."""
    out = []
    while s > 768:
        out.append(512)
        s -= 512
    if s > 512:
        out.append(s - 256)
        out.append(256)
    elif s > 0:
        out.append(s)
    return out


def _pairs(C):
    """Column ranges grouped <=2 parts each, so G12 holds at most
    4 PSUM banks (2 parts x gate+up) at a time."""
    parts = _parts(C)
    groups = []
    o = 0
    for i in range(0, len(parts), 2):
        ps = parts[i:i + 2]
        offs = []
        for p_ in ps:
            offs.append((o, p_))
            o += p_
        groups.append(offs)
    return groups


def _build_nc(caps):
    import concourse.tile as tile
    from concourse import bacc, mybir

    nc = bacc.Bacc("TRN2", target_bir_lowering=False, debug=False,
                   enable_asserts=False, num_devices=N_CORES)
    dt = mybir.dt.float32
    mdt = mybir.dt.bfloat16
    SILU = mybir.ActivationFunctionType.Silu

    xts, w1ts, v1ts, w2s, ys = [], [], [], [], []
    for s in range(N_SLOTS):
        C = caps[s]
        xts.append(nc.dram_tensor(f"xt{s}", [H, C], mdt,
                                  kind="ExternalInput").ap())
        # w1t/v1t pre-blocked: [ft, p(h%128), o(h//128), f] so each ft
        # slice is contiguous and DMAs as 128 x 2KB descriptors
        w1ts.append(nc.dram_tensor(f"w1t{s}", [FT, P, HT, P], mdt,
                                   kind="ExternalInput").ap())
        v1ts.append(nc.dram_tensor(f"v1t{s}", [FT, P, HT, P], mdt,
                                   kind="ExternalInput").ap())
        w2s.append(nc.dram_tensor(f"w2_{s}", [F, H], mdt,
                                  kind="ExternalInput").ap())
        # y in [H, C] layout: tokens are GEMM3's MOVING dim, so the
        # token count needs no 128-padding (host transposes on combine)
        ys.append(nc.dram_tensor(f"y{s}", [H, C], dt,
                                 kind="ExternalOutput").ap())

    with tile.TileContext(nc) as tc:
        with ExitStack() as ctx:
            # 16 xt bufs: both slots' tiles resident so slot1's DMA
            # streams during slot0 compute
            xt_pool = ctx.enter_context(tc.tile_pool(name="xt", bufs=2 * HT))
            wst_pool = ctx.enter_context(tc.tile_pool(name="wst", bufs=5))
            w2_pool = ctx.enter_context(tc.tile_pool(name="w2sb", bufs=FT))
            hact_pool = ctx.enter_context(tc.tile_pool(name="hact", bufs=FT))
            silu_pool = ctx.enter_context(tc.tile_pool(name="silu", bufs=3))
            out_pool = ctx.enter_context(tc.tile_pool(name="out", bufs=2))
            ps_pool = ctx.enter_context(tc.tile_pool(name="ps", bufs=8, space="PSUM"))

            # PE p-state warmup: the first ~3us of continuous PE execution
            # run at reduced clock (window-0 matmuls measure ~346ns vs 216
            # steady = ~2.7us one-time tax). Burn the ramp on dummy matmuls
            # over a zeroed scratch tile during the otherwise-idle startup
            # DMA fill (~12.6us); sized to end just before real work.
            scr = wst_pool.tile([P, 512], mdt, tag="wst", name="warm_scr")
            nc.vector.memset(scr[:], 0.0)
            d_scr = ps_pool.tile([P, 512], dt, tag="ps", name="warm_ps")
            NW = 52
            for i in range(NW):
                nc.tensor.matmul(d_scr[:], scr[:, 0:P], scr[:],
                                 start=(i == 0), stop=(i == NW - 1))

            for s in range(N_SLOTS):
                C = caps[s]
                xt, w1t, v1t, w2, y = xts[s], w1ts[s], v1ts[s], w2s[s], ys[s]
                groups = _pairs(C)

                # ft0 weights land before the xt block: the first matmul
                # waits on ~0.75MB of DMA, then consumes xt ht-by-ht
                w1s0 = wst_pool.tile([P, HT, P], mdt, tag="wst")
                v1s0 = wst_pool.tile([P, HT, P], mdt, tag="wst")
                xt_sb = [xt_pool.tile([P, C], mdt, tag="xt",
                                      name=f"xt_sb{ht}")
                         for ht in range(HT)]
                # issue order = first-matmul dependency order (startup is
                # DMA-bandwidth-bound; head-splitting and dual-engine
                # issue variants all measured worse or neutral)
                nc.sync.dma_start(w1s0[:], w1t[0])
                nc.sync.dma_start(xt_sb[0][:], xt[0:P, :])
                nc.sync.dma_start(v1s0[:], v1t[0])
                for ht in range(1, HT):
                    nc.sync.dma_start(xt_sb[ht][:],
                                      xt[ht * P:(ht + 1) * P, :])

                # GEMM1/2 + GLU, ft-outer: w1/v1 stream exactly once
                hact_sb = []
                w2_sb = []
                for ft in range(FT):
                    h_t = hact_pool.tile([P, C], mdt, tag="hact")
                    if ft == 0:
                        w1s, v1s = w1s0, v1s0
                    else:
                        w1s = wst_pool.tile([P, HT, P], mdt, tag="wst")
                        v1s = wst_pool.tile([P, HT, P], mdt, tag="wst")
                        nc.sync.dma_start(w1s[:], w1t[ft])
                        nc.sync.dma_start(v1s[:], v1t[ft])
                    for offs in groups:
                        g_tiles = [ps_pool.tile([P, p_], dt, tag="ps",
                                                name=f"g{i_}")
                                   for i_, (_, p_) in enumerate(offs)]
                        u_tiles = [ps_pool.tile([P, p_], dt, tag="ps",
                                               name=f"u{i_}")
                                   for i_, (_, p_) in enumerate(offs)]
                        for ht in range(HT):
                            for i_, (o_, p_) in enumerate(offs):
                                nc.tensor.matmul(
                                    g_tiles[i_][:], w1s[:, ht, :],
                                    xt_sb[ht][:, o_:o_ + p_],
                                    start=(ht == 0), stop=(ht == HT - 1))
                            for i_, (o_, p_) in enumerate(offs):
                                nc.tensor.matmul(
                                    u_tiles[i_][:], v1s[:, ht, :],
                                    xt_sb[ht][:, o_:o_ + p_],
                                    start=(ht == 0), stop=(ht == HT - 1))
                        for i_, (o_, p_) in enumerate(offs):
                            sl = silu_pool.tile([P, p_], dt, tag="sl")
                            nc.scalar.activation(sl[:], g_tiles[i_][:], SILU)
                            nc.vector.tensor_mul(
                                h_t[:, o_:o_ + p_], sl[:], u_tiles[i_][:])
                    hact_sb.append(h_t)
                    # trickle w2 in, 2 tiles per ft iteration: a bulk
                    # issue after ft0 queued 4MB ahead of ft1's weight
                    # tiles during the startup xt fill and stalled the
                    # PE ~5us; spread issuance never blocks the stream
                    if ft < FT // 2:
                        for f2 in (2 * ft, 2 * ft + 1):
                            t = w2_pool.tile([P, H], mdt, tag="w2",
                                             name=f"w2_{f2}")
                            nc.sync.dma_start(
                                t[:], w2[f2 * P:(f2 + 1) * P, :])
                            w2_sb.append(t)

                # GEMM3: down_T[h, c] = sum_f w2[f,h] * hact[f,c].
                # Stationary = w2 [128f, 128h], moving = hact column part
                # (tokens never padded to 128). Each accumulation group's
                # 16 matmuls stay contiguous on one PSUM bank (alternating
                # banks under one stationary costs ~120ns/MM).
                fparts = []
                fo = 0
                for p_ in _parts(C):
                    fparts.append((fo, p_))
                    fo += p_
                for ht2 in range(HT):
                    o_t = out_pool.tile([P, C], dt, tag="o")
                    for oi, (o_, p_) in enumerate(fparts):
                        d_ps = ps_pool.tile([P, p_], dt, tag="ps",
                                            name=f"d{oi % 2}")
                        for ft in range(FT):
                            nc.tensor.matmul(
                                d_ps[:],
                                w2_sb[ft][:, ht2 * P:(ht2 + 1) * P],
                                hact_sb[ft][:, o_:o_ + p_],
                                start=(ft == 0), stop=(ft == FT - 1))
                        nc.any.tensor_copy(o_t[:, o_:o_ + p_], d_ps[:])
                        nc.sync.dma_start(
                            y[ht2 * P:(ht2 + 1) * P, o_:o_ + p_],
                            o_t[:, o_:o_ + p_])
    nc.compile()
    return nc


def _get_nc(caps):
    if caps not in _nc_cache:
        _nc_cache[caps] = _build_nc(caps)
    return _nc_cache[caps]


def prepare(x, top_weights, top_experts, w1, v1, w2):
    """Host-side routing + sharded input construction."""
    import ml_dtypes
    bf16 = ml_dtypes.bfloat16
    x = np.asarray(x, dtype=np.float32)
    top_weights = np.asarray(top_weights, dtype=np.float32)
    top_experts = np.asarray(top_experts).astype(np.int64)

    xf = x.reshape(T, H)

    # combine weights per (token, expert); duplicate slots sum
    cw = np.zeros((T, E), dtype=np.float32)
    np.add.at(cw, (np.arange(T)[:, None], top_experts), top_weights)

    idx = [np.nonzero(cw[:, e])[0] for e in range(E)]
    counts = [len(i) for i in idx]

    order = sorted(range(E), key=lambda e: -counts[e])
    slot_experts = [[order[m] for m in range(N_CORES)],
                    [order[2 * N_CORES - 1 - m] for m in range(N_CORES)]]
    # even capacities keep all matmul free sizes even (ISA-safe)
    caps = tuple(max(256, (max(counts[e] for e in slot_experts[s]) + 1)
                 // 2 * 2) for s in range(N_SLOTS))

    def _block(we):
        # [F, H] -> [ft, p(h%128), o(h//128), f], bf16
        wl = np.asarray(we, dtype=np.float32).reshape(FT, P, HT, P)
        return np.ascontiguousarray(
            wl.transpose(0, 3, 2, 1)).astype(bf16)

    in_maps = []
    for m in range(N_CORES):
        im = {}
        for s in range(N_SLOTS):
            e = slot_experts[s][m]
            C = caps[s]
            XT = np.zeros((H, C), dtype=bf16)
            XT[:, :counts[e]] = xf[idx[e]].T.astype(bf16)
            im[f"xt{s}"] = XT
            im[f"w1t{s}"] = _block(w1[e])
            im[f"v1t{s}"] = _block(v1[e])
            im[f"w2_{s}"] = np.ascontiguousarray(
                np.asarray(w2[e], dtype=np.float32)).astype(bf16)
        in_maps.append(im)
    return caps, in_maps, slot_experts, idx, counts, cw


def combine(results, slot_experts, idx, counts, cw):
    """Weighted scatter-add of per-core expert outputs into [B, S, H]."""
    out = np.zeros((T, H), dtype=np.float32)
    for m in range(N_CORES):
        for s in range(N_SLOTS):
            e = slot_experts[s][m]
            n = counts[e]
            if n:
                ym = results[m][f"y{s}"]  # [H, C]
                out[idx[e]] += ym[:, :n].T * cw[idx[e], e][:, None]
    return out.reshape(B, S, H)


def kernel(x, weights, top_weights, top_experts, w1, v1, w2):
    global LAST_RESULT
    caps, in_maps, slot_experts, idx, counts, cw = prepare(
        x, top_weights, top_experts, w1, v1, w2)
    nc = _get_nc(caps)
    from concourse.bass_utils import run_bass_kernel_spmd
    res = run_bass_kernel_spmd(nc, in_maps, list(range(N_CORES)), trace=TRACE,
                               trace_cores=TRACE_CORES if TRACE else None)
    LAST_RESULT = res
    return combine(res.results, slot_experts, idx, counts, cw)


# revision 17
# speedup vs baseline: 1.0094x; 1.0094x over previous
"""MoE (DbrxExperts) expert-parallel Trainium2 kernel, v3 (bf16, ft-outer).

Strategy:
  - Host: route tokens to experts, sort experts by count desc, rank-pair
    onto cores (core m: slot0 = rank[m], slot1 = rank[15-m]). Slot
    capacities C0/C1 are exact maxima, baked into the program.
  - All matmul operands bf16 (PE rate identical to fp32r; DMA halved;
    whole slot's xt + hact fit in SBUF). fp32 accumulation in PSUM.
  - Device, per slot (ft-outer so w1/v1 stream exactly once):
      xt (8 tiles [128, C]) DMA'd up front, consumed ht-progressively.
      for ft: load w1s/v1s [128, HT*128]; for each column-pair group:
        gate/up accumulate over ht in PSUM; silu (ACT) * up (DVE) ->
        hact[ft] bf16 in SBUF.
      for ht2 (8 h-tiles): for token-part (<=512): down_T[h, c]
        accumulated over 16 ft in PSUM -> copy -> DMA y [H, C] fp32
        (tokens moving => no 128-padding of the token dim).
  - Host: out[tokens_e] += down_e.T * cw_e.
"""

import numpy as np
from contextlib import ExitStack

N_CORES = 8
B, S, H = 4, 2048, 1024
F, E = 2048, 16
T = B * S
N_SLOTS = 2  # experts per core

P = 128
HT = H // P   # 8  h-tiles
FT = F // P   # 16 f-tiles

TRACE = False          # test.py sets this for profiled runs
TRACE_CORES = [7]      # core-0 NTFF capture crashes fast kernels here
LAST_RESULT = None     # BassKernelResults of last run (for test.py)

_nc_cache = {}


def _parts(s):
    """Split s columns into matmul parts, each <=512 and >=256 where
    possible (moving dim >=256 keeps full PE rate)."""
    out = []
    while s > 768:
        out.append(512)
        s -= 512
    if s > 512:
        out.append(s - 256)
        out.append(256)
    elif s > 0:
        out.append(s)
    return out


def _pairs(C):
    """Column ranges grouped <=2 parts each, so G12 holds at most
    4 PSUM banks (2 parts x gate+up) at a time."""
    parts = _parts(C)
    groups = []
    o = 0
    for i in range(0, len(parts), 2):
        ps = parts[i:i + 2]
        offs = []
        for p_ in ps:
            offs.append((o, p_))
            o += p_
        groups.append(offs)
    return groups


def _build_nc(caps):
    import concourse.tile as tile
    from concourse import bacc, mybir

    nc = bacc.Bacc("TRN2", target_bir_lowering=False, debug=False,
                   enable_asserts=False, num_devices=N_CORES)
    dt = mybir.dt.float32
    mdt = mybir.dt.bfloat16
    SILU = mybir.ActivationFunctionType.Silu

    xts, w1ts, v1ts, w2s, ys = [], [], [], [], []
    for s in range(N_SLOTS):
        C = caps[s]
        xts.append(nc.dram_tensor(f"xt{s}", [H, C], mdt,
                                  kind="ExternalInput").ap())
        # w1t/v1t pre-blocked: [ft, p(h%128), o(h//128), f] so each ft
        # slice is contiguous and DMAs as 128 x 2KB descriptors
        w1ts.append(nc.dram_tensor(f"w1t{s}", [FT, P, HT, P], mdt,
                                   kind="ExternalInput").ap())
        v1ts.append(nc.dram_tensor(f"v1t{s}", [FT, P, HT, P], mdt,
                                   kind="ExternalInput").ap())
        w2s.append(nc.dram_tensor(f"w2_{s}", [F, H], mdt,
                                  kind="ExternalInput").ap())
        # y in [H, C] layout: tokens are GEMM3's MOVING dim, so the
        # token count needs no 128-padding (host transposes on combine)
        ys.append(nc.dram_tensor(f"y{s}", [H, C], dt,
                                 kind="ExternalOutput").ap())

    with tile.TileContext(nc) as tc:
        with ExitStack() as ctx:
            # 16 xt bufs: both slots' tiles resident so slot1's DMA
            # streams during slot0 compute
            xt_pool = ctx.enter_context(tc.tile_pool(name="xt", bufs=2 * HT))
            wst_pool = ctx.enter_context(tc.tile_pool(name="wst", bufs=4))
            w2_pool = ctx.enter_context(tc.tile_pool(name="w2sb", bufs=FT))
            hact_pool = ctx.enter_context(tc.tile_pool(name="hact", bufs=FT))
            silu_pool = ctx.enter_context(tc.tile_pool(name="silu", bufs=3))
            out_pool = ctx.enter_context(tc.tile_pool(name="out", bufs=2))
            ps_pool = ctx.enter_context(tc.tile_pool(name="ps", bufs=8, space="PSUM"))

            # (A p-state warmup with dummy matmuls during the startup DMA
            # fill measured 5.3us WORSE: the dummy chain itself runs at
            # ramped clock and delays real work without the DVFS credit.)
            for s in range(N_SLOTS):
                C = caps[s]
                xt, w1t, v1t, w2, y = xts[s], w1ts[s], v1ts[s], w2s[s], ys[s]
                groups = _pairs(C)

                # ft0 weights land before the xt block: the first matmul
                # waits on ~0.75MB of DMA, then consumes xt ht-by-ht
                w1s0 = wst_pool.tile([P, HT, P], mdt, tag="wst")
                v1s0 = wst_pool.tile([P, HT, P], mdt, tag="wst")
                xt_sb = [xt_pool.tile([P, C], mdt, tag="xt",
                                      name=f"xt_sb{ht}")
                         for ht in range(HT)]
                # issue order = first-matmul dependency order (startup is
                # DMA-bandwidth-bound; head-splitting and dual-engine
                # issue variants all measured worse or neutral)
                nc.sync.dma_start(w1s0[:], w1t[0])
                nc.sync.dma_start(xt_sb[0][:], xt[0:P, :])
                nc.sync.dma_start(v1s0[:], v1t[0])
                for ht in range(1, HT):
                    nc.sync.dma_start(xt_sb[ht][:],
                                      xt[ht * P:(ht + 1) * P, :])

                # GEMM1/2 + GLU, ft-outer: w1/v1 stream exactly once
                hact_sb = []
                w2_sb = []
                for ft in range(FT):
                    h_t = hact_pool.tile([P, C], mdt, tag="hact")
                    if ft == 0:
                        w1s, v1s = w1s0, v1s0
                    else:
                        w1s = wst_pool.tile([P, HT, P], mdt, tag="wst")
                        v1s = wst_pool.tile([P, HT, P], mdt, tag="wst")
                        nc.sync.dma_start(w1s[:], w1t[ft])
                        nc.sync.dma_start(v1s[:], v1t[ft])
                    for offs in groups:
                        g_tiles = [ps_pool.tile([P, p_], dt, tag="ps",
                                                name=f"g{i_}")
                                   for i_, (_, p_) in enumerate(offs)]
                        u_tiles = [ps_pool.tile([P, p_], dt, tag="ps",
                                               name=f"u{i_}")
                                   for i_, (_, p_) in enumerate(offs)]
                        for ht in range(HT):
                            for i_, (o_, p_) in enumerate(offs):
                                nc.tensor.matmul(
                                    g_tiles[i_][:], w1s[:, ht, :],
                                    xt_sb[ht][:, o_:o_ + p_],
                                    start=(ht == 0), stop=(ht == HT - 1))
                            for i_, (o_, p_) in enumerate(offs):
                                nc.tensor.matmul(
                                    u_tiles[i_][:], v1s[:, ht, :],
                                    xt_sb[ht][:, o_:o_ + p_],
                                    start=(ht == 0), stop=(ht == HT - 1))
                        for i_, (o_, p_) in enumerate(offs):
                            sl = silu_pool.tile([P, p_], dt, tag="sl")
                            nc.scalar.activation(sl[:], g_tiles[i_][:], SILU)
                            nc.vector.tensor_mul(
                                h_t[:, o_:o_ + p_], sl[:], u_tiles[i_][:])
                    hact_sb.append(h_t)
                    # trickle w2 in, 2 tiles per ft iteration: a bulk
                    # issue after ft0 queued 4MB ahead of ft1's weight
                    # tiles during the startup xt fill and stalled the
                    # PE ~5us; spread issuance never blocks the stream
                    if ft < FT // 2:
                        for f2 in (2 * ft, 2 * ft + 1):
                            t = w2_pool.tile([P, H], mdt, tag="w2",
                                             name=f"w2_{f2}")
                            nc.sync.dma_start(
                                t[:], w2[f2 * P:(f2 + 1) * P, :])
                            w2_sb.append(t)

                # GEMM3: down_T[h, c] = sum_f w2[f,h] * hact[f,c].
                # Stationary = w2 [128f, 128h], moving = hact column part
                # (tokens never padded to 128). Each accumulation group's
                # 16 matmuls stay contiguous on one PSUM bank (alternating
                # banks under one stationary costs ~120ns/MM).
                fparts = []
                fo = 0
                for p_ in _parts(C):
                    fparts.append((fo, p_))
                    fo += p_
                for ht2 in range(HT):
                    o_t = out_pool.tile([P, C], dt, tag="o")
                    for oi, (o_, p_) in enumerate(fparts):
                        d_ps = ps_pool.tile([P, p_], dt, tag="ps",
                                            name=f"d{oi % 2}")
                        for ft in range(FT):
                            nc.tensor.matmul(
                                d_ps[:],
                                w2_sb[ft][:, ht2 * P:(ht2 + 1) * P],
                                hact_sb[ft][:, o_:o_ + p_],
                                start=(ft == 0), stop=(ft == FT - 1))
                        nc.any.tensor_copy(o_t[:, o_:o_ + p_], d_ps[:])
                        nc.sync.dma_start(
                            y[ht2 * P:(ht2 + 1) * P, o_:o_ + p_],
                            o_t[:, o_:o_ + p_])
    nc.compile()
    return nc


def _get_nc(caps):
    if caps not in _nc_cache:
        _nc_cache[caps] = _build_nc(caps)
    return _nc_cache[caps]


def prepare(x, top_weights, top_experts, w1, v1, w2):
    """Host-side routing + sharded input construction."""
    import ml_dtypes
    bf16 = ml_dtypes.bfloat16
    x = np.asarray(x, dtype=np.float32)
    top_weights = np.asarray(top_weights, dtype=np.float32)
    top_experts = np.asarray(top_experts).astype(np.int64)

    xf = x.reshape(T, H)

    # combine weights per (token, expert); duplicate slots sum
    cw = np.zeros((T, E), dtype=np.float32)
    np.add.at(cw, (np.arange(T)[:, None], top_experts), top_weights)

    idx = [np.nonzero(cw[:, e])[0] for e in range(E)]
    counts = [len(i) for i in idx]

    order = sorted(range(E), key=lambda e: -counts[e])
    slot_experts = [[order[m] for m in range(N_CORES)],
                    [order[2 * N_CORES - 1 - m] for m in range(N_CORES)]]
    # even capacities keep all matmul free sizes even (ISA-safe)
    caps = tuple(max(256, (max(counts[e] for e in slot_experts[s]) + 1)
                 // 2 * 2) for s in range(N_SLOTS))

    def _block(we):
        # [F, H] -> [ft, p(h%128), o(h//128), f], bf16
        wl = np.asarray(we, dtype=np.float32).reshape(FT, P, HT, P)
        return np.ascontiguousarray(
            wl.transpose(0, 3, 2, 1)).astype(bf16)

    in_maps = []
    for m in range(N_CORES):
        im = {}
        for s in range(N_SLOTS):
            e = slot_experts[s][m]
            C = caps[s]
            XT = np.zeros((H, C), dtype=bf16)
            XT[:, :counts[e]] = xf[idx[e]].T.astype(bf16)
            im[f"xt{s}"] = XT
            im[f"w1t{s}"] = _block(w1[e])
            im[f"v1t{s}"] = _block(v1[e])
            im[f"w2_{s}"] = np.ascontiguousarray(
                np.asarray(w2[e], dtype=np.float32)).astype(bf16)
        in_maps.append(im)
    return caps, in_maps, slot_experts, idx, counts, cw


def combine(results, slot_experts, idx, counts, cw):
    """Weighted scatter-add of per-core expert outputs into [B, S, H]."""
    out = np.zeros((T, H), dtype=np.float32)
    for m in range(N_CORES):
        for s in range(N_SLOTS):
            e = slot_experts[s][m]
            n = counts[e]
            if n:
                ym = results[m][f"y{s}"]  # [H, C]
                out[idx[e]] += ym[:, :n].T * cw[idx[e], e][:, None]
    return out.reshape(B, S, H)


def kernel(x, weights, top_weights, top_experts, w1, v1, w2):
    global LAST_RESULT
    caps, in_maps, slot_experts, idx, counts, cw = prepare(
        x, top_weights, top_experts, w1, v1, w2)
    nc = _get_nc(caps)
    from concourse.bass_utils import run_bass_kernel_spmd
    res = run_bass_kernel_spmd(nc, in_maps, list(range(N_CORES)), trace=TRACE,
                               trace_cores=TRACE_CORES if TRACE else None)
    LAST_RESULT = res
    return combine(res.results, slot_experts, idx, counts, cw)
